# revision 41
# baseline (speedup 1.0000x reference)
"""Multi-head self-attention on 8 Trainium2 NeuronCores.

Problem: x[2, 2048, 1024], 16 heads, Dh=64, fp32.
  q/k/v = x @ W.T ; scores = q k^T / 8 ; out = softmax(scores) v @ W_o.T

Sharding (Megatron-style): each core owns 2 heads (128 of the 1024 model
dims). W_q/W_k/W_v column-sharded, W_o row-sharded; the cross-core
all-reduce of the output-projection partials is done on the host.

Per-core design (v2, all-bf16 matmuls):
  - Every matmul operand is bf16: 1 column/cycle on the PE at 2.4 GHz and
    fast-weight-load stays enabled (fp32/fp32r HIGH mode disables FWL and
    measured ~1.3-1.5 cyc/col). PSUM accumulation is fp32 throughout.
  - x is fed pre-transposed (xT [1024, 4096] bf16); one 3D-pattern DMA per
    512-token block gathers all 8 contraction strips into one SBUF tile.
  - qT/kT computed as [128 = 2 heads x 64, 4096] (head dims on partitions).
  - Scores computed transposed, S^T[k_tok, q_tok], both heads in one
    [128, 1024] 2-bank PSUM tile per k-strip; exp() on ScalarE reads PSUM
    directly (PSUM access is cheaper than SBUF for ACT) and writes bf16.
  - v transposed to natural layout via PE transpose, augmented with a ones
    column so the PV matmul (M=65) emits softmax denominators in the same
    accumulation chain.
  - Denominator path stays on-chip: DVE reciprocal (fp22) on the PSUM
    denominator rows, one tiny SBUF->SBUF DMA to rebase the two rows onto
    partitions 0/1, then a single K=2 matmul broadcasts 1/den across all
    128 partitions for the normalization multiply.
  - Output written bf16; the host all-reduce upcasts to fp32.
  - Software-pipelined schedule: ScalarE runs one exp [128,1024] per
    k-strip back-to-back; all PE work (scores, PV, QKV chains, transposes,
    output projection) is interleaved as filler chunks between strips.
"""

import os
from contextlib import ExitStack

import numpy as np

import concourse.bass as bass
import concourse.tile as tile
from concourse import bacc, mybir
from concourse._compat import with_exitstack
from concourse.bass_utils import run_bass_kernel_spmd

F32 = mybir.dt.float32
F32R = mybir.dt.float32r
BF = mybir.dt.bfloat16
EXP = mybir.ActivationFunctionType.Exp

P = 128          # partitions / head-pair dims per core
D = 1024         # model dim
T = 2048         # tokens per batch
NB = 2           # batches
BT = NB * T      # 4096 flattened tokens
KT = D // P      # 8 contraction tiles over model dim
NQ = T // 512    # 4 q-tiles of 512 per batch
NS = T // P      # 16 k-strips of 128 per batch
N_CORES = 8


@with_exitstack
def _mhsa_kernel(ctx: ExitStack, tc: tile.TileContext, out, xTb, wqT, wkT,
                 wvT, woT, ident_in, ones_in, sel_in):
    nc = tc.nc

    # ---- pools ----
    wpool = ctx.enter_context(tc.tile_pool(name="weights", bufs=1))
    xpool = ctx.enter_context(tc.tile_pool(name="xtiles", bufs=3))
    qkpool = ctx.enter_context(tc.tile_pool(name="qk", bufs=1))
    vtpool = ctx.enter_context(tc.tile_pool(name="vt", bufs=2))
    vapool = ctx.enter_context(tc.tile_pool(name="vaug", bufs=4))
    expool = ctx.enter_context(tc.tile_pool(name="expp", bufs=4))
    aupool = ctx.enter_context(tc.tile_pool(name="avun", bufs=3))
    anpool = ctx.enter_context(tc.tile_pool(name="avnorm", bufs=3))
    rpool = ctx.enter_context(tc.tile_pool(name="recip", bufs=2))
    opool = ctx.enter_context(tc.tile_pool(name="outsb", bufs=4))
    drpool = ctx.enter_context(tc.tile_pool(name="dscr", bufs=2, space="DRAM"))

    ps_sc = ctx.enter_context(tc.tile_pool(name="ps_sc", bufs=2, space="PSUM"))
    ps_pv = ctx.enter_context(tc.tile_pool(name="ps_pv", bufs=2, space="PSUM"))
    ps_misc = ctx.enter_context(tc.tile_pool(name="ps_misc", bufs=2,
                                             space="PSUM"))

    # ---- weights / constants (resident) ----
    # One DMA per projection: wqT [1024, 128] -> [128, 8*128] with the 8
    # contraction strips side by side on the free axis.
    wq_sb = wpool.tile([P, KT * P], BF, name="wq_sb")
    wk_sb = wpool.tile([P, KT * P], BF, name="wk_sb")
    wv_sb = wpool.tile([P, KT * P], BF, name="wv_sb")
    wo_sb = wpool.tile([P, D], BF, name="wo_sb")
    ident = wpool.tile([P, P], F32, name="ident")
    sel2 = wpool.tile([2, P], F32R, name="sel2")
    for eng, w_sb, w_in in ((nc.sync, wq_sb, wqT), (nc.gpsimd, wk_sb, wkT),
                            (nc.sync, wv_sb, wvT)):
        eng.dma_start(
            out=w_sb[:].rearrange("p (j m) -> p j m", j=KT),
            in_=w_in.rearrange("(j p) m -> p j m", p=P),
        )
    nc.gpsimd.dma_start(out=wo_sb[:], in_=woT[:])
    nc.sync.dma_start(out=ident[:], in_=ident_in[:])
    nc.gpsimd.dma_start(out=sel2[:], in_=sel_in[:])

    qT = qkpool.tile([P, BT], BF, name="qT")
    kTt = qkpool.tile([P, BT], BF, name="kTt")

    va = {}       # (b, 'A'/'B') -> augmented-v tile, (b, 'T') -> vT staging
    avs = {}      # (b, n, 'A'/'B') -> PV psum accumulators
    xts = {}      # (b, n) -> x strip tile [128, 8*512]

    def emit_qkv_chunks(b, inline_n0=False):
        """Chunk list: x loads, QKV chains, v transposes for batch b.

        Chains are split in two 4-matmul halves so a single chunk never
        occupies the PE for more than ~1us between attention strips.
        """
        chunks = []

        def init():
            vA = vapool.tile([P, NS * 65], BF, name=f"vA{b}", tag="va")
            vB = vapool.tile([P, NS * 65], BF, name=f"vB{b}", tag="va")
            nc.gpsimd.dma_start(out=vA[:], in_=ones_in[:])
            nc.gpsimd.dma_start(out=vB[:], in_=ones_in[:])
            va[(b, "A")] = vA
            va[(b, "B")] = vB
            vT = vtpool.tile([P, T], F32, name=f"vT{b}", tag="vt")
            va[(b, "T")] = vT
        chunks.append(init)

        def load_n(n):
            def f():
                col = b * T + n * 512
                xt = xpool.tile([P, KT * 512], BF, name=f"xt{b}_{n}", tag="xt")
                nc.sync.dma_start(
                    out=xt[:].rearrange("p (j c) -> p j c", j=KT),
                    in_=xTb.rearrange("(j p) c -> p j c", p=P)[:, :, col:col + 512],
                )
                xts[(b, n)] = xt
            return f

        def chain(n, which):
            # One chunk per chain: every reader of the PSUM accumulator is
            # emitted in-chunk, so interleaved chunks can never recycle it
            # out from under a not-yet-emitted reader.
            def f():
                acc = ps_misc.tile([P, 512], F32, name="qkv_ps", tag="mps")
                w_sb = {"q": wq_sb, "k": wk_sb, "v": wv_sb}[which]
                xt = xts[(b, n)]
                for j in range(KT):
                    nc.tensor.matmul(
                        acc[:], w_sb[:, j * P:(j + 1) * P],
                        xt[:, j * 512:(j + 1) * 512],
                        start=(j == 0), stop=(j == KT - 1),
                    )
                dst, col = {
                    "q": (qT, b * T + n * 512),
                    "k": (kTt, b * T + n * 512),
                    "v": (va[(b, "T")], n * 512),
                }[which]
                nc.vector.tensor_copy(dst[:, col:col + 512], acc[:])
            return f

        def trans(s):
            def f():
                vT = va[(b, "T")]
                tp = ps_misc.tile([P, P], F32, name="tr_ps", tag="mps")
                nc.tensor.transpose(tp[:], vT[:, s * P:(s + 1) * P], ident[:])
                nc.vector.tensor_copy(
                    va[(b, "A")][:, s * 65:s * 65 + 64], tp[:, 0:64])
                nc.vector.tensor_copy(
                    va[(b, "B")][:, s * 65:s * 65 + 64], tp[:, 64:128])
            return f

        # Per-n interleave keeps the 3-deep x-tile pool cycling: all of
        # xt(n)'s readers (k/v/q chains) are emitted before load(n+3)
        # needs its slot back, so the in-order queues can't cycle.
        chunks.append(init)
        for n in range(NQ):
            chunks.append(load_n(n))
            for which in ("k", "v", "q"):
                chunks.append(chain(n, which))
            for s in range(4 * n, 4 * n + 4):
                chunks.append(trans(s))
        return chunks

    def make_norm_outproj(b):
        """Per-q-tile deferred closure lists: normalize + project.

        The PSUM readers (av_un copies, reciprocals) are emitted INLINE at
        the end of each q-tile by `den_inline` — deferring them past the
        next q-tile's PSUM-tile allocations would hand their buffers out
        before the reads are visible to the dependency tracker.
        """
        state = {}

        def den_inline(n):
            avA = avs[(b, n, "A")]
            avB = avs[(b, n, "B")]
            av_un = aupool.tile([P, 512], BF, name="av_un", tag="aun")
            nc.vector.tensor_copy(av_un[0:64, :], avA[0:64, :])
            nc.vector.tensor_copy(av_un[64:128, :], avB[0:64, :])
            rr = rpool.tile([P, 1024], F32R, name="rr", tag="rr")
            with nc.allow_low_precision(reason="fp22 softmax denom"):
                nc.vector.reciprocal(rr[64:65, 0:512], avA[64:65, :])
                nc.vector.reciprocal(rr[64:65, 512:1024], avB[64:65, :])
            dscr = drpool.tile([1, 1024], F32R, name="dscr", tag="dscr")
            bc2 = rpool.tile([2, 512], F32R, name="bc2", tag="bc2")
            nc.sync.dma_start(out=dscr[:], in_=rr[64:65, :])
            nc.sync.dma_start(
                out=bc2[:], in_=dscr.rearrange("p (a c) -> (p a) c", a=2))
            state[(n, "un")] = av_un
            state[(n, "bc")] = bc2

        def norm(n):
            def f():
                bcp = ps_misc.tile([P, 512], F32, name="bc_ps", tag="mps")
                nc.tensor.matmul(bcp[:], sel2[:], state[(n, "bc")][:],
                                 start=True, stop=True)
                av_n = anpool.tile([P, 512], BF, name="av_n", tag="avn")
                nc.vector.tensor_mul(av_n[:], state[(n, "un")][:], bcp[:])
                state[(n, "nrm")] = av_n
            return f

        def outproj(n, sub):
            def f():
                av_n = state[(n, "nrm")]
                row0 = b * T + n * 512 + sub * P
                for jh in range(2):
                    op = ps_misc.tile([P, 512], F32, name="op_ps", tag="mps")
                    nc.tensor.matmul(
                        op[:],
                        av_n[:, sub * P:(sub + 1) * P],
                        wo_sb[:, jh * 512:(jh + 1) * 512],
                        start=True, stop=True,
                    )
                    ot = opool.tile([P, 512], BF, name="ot", tag="ot")
                    nc.vector.tensor_copy(ot[:], op[:])
                    eng = nc.sync if (sub + jh) % 2 == 0 else nc.gpsimd
                    eng.dma_start(
                        out=out[row0:row0 + P, jh * 512:(jh + 1) * 512],
                        in_=ot[:],
                    )
            return f

        def per_n(n):
            return [norm(n)] + [outproj(n, sub) for sub in range(4)]
        return den_inline, per_n

    def attn_batch(b, queue, urgent, self_np=None):
        """Batch-b attention; filler chunks drained between strip steps.

        `urgent` chunks (deferred norm/outproj of a finished q-tile) pop
        ahead of regular fillers so their tile-pool buffers recycle on
        schedule. Leftovers are the caller's to drain.
        """
        den_inline = None
        if self_np is not None:
            den_inline, per_n = self_np
        for n in range(NQ):
            qcol = b * T + n * 512
            avA = ps_pv.tile([P, 512], F32, name="avA", tag="pv")
            avB = ps_pv.tile([P, 512], F32, name="avB", tag="pv")
            avs[(b, n, "A")] = avA
            avs[(b, n, "B")] = avB
            for s in range(NS):
                kcol = b * T + s * P
                sc = ps_sc.tile([P, 1024], F32, name="sc", tag="sc")
                nc.tensor.matmul(
                    sc[:, 0:512],
                    kTt[0:64, kcol:kcol + P],
                    qT[0:64, qcol:qcol + 512],
                    start=True, stop=True,
                )
                nc.tensor.matmul(
                    sc[:, 512:1024],
                    kTt[64:128, kcol:kcol + P],
                    qT[64:128, qcol:qcol + 512],
                    start=True, stop=True,
                )
                ex = expool.tile([P, 1024], BF, name="ex", tag="ex")
                nc.scalar.activation(out=ex[:], in_=sc[:], func=EXP,
                                     scale=0.125)
                # PE filler work goes here, during the exp wait.
                if urgent:
                    urgent.popleft()()
                elif queue:
                    queue.popleft()()
                nc.tensor.matmul(
                    avA[0:65, :],
                    va[(b, "A")][:, s * 65:(s + 1) * 65],
                    ex[:, 0:512],
                    start=(s == 0), stop=(s == NS - 1),
                )
                nc.tensor.matmul(
                    avB[0:65, :],
                    va[(b, "B")][:, s * 65:(s + 1) * 65],
                    ex[:, 512:1024],
                    start=(s == 0), stop=(s == NS - 1),
                )
            if self_np is not None:
                den_inline(n)
                urgent.extend(per_n(n))

    # ---- software-pipelined schedule ----
    from collections import deque
    chunks0 = emit_qkv_chunks(0)
    # Inline: everything attn(b0) strips read (x, k, v, q, transposes).
    # Batch 1's chunks become the fillers inside batch 0's attention.
    for c in chunks0:
        c()
    np0 = make_norm_outproj(0)
    np1 = make_norm_outproj(1)
    urgent = deque()
    queue = deque(emit_qkv_chunks(1))
    attn_batch(0, queue, urgent, self_np=np0)
    attn_batch(1, queue, urgent, self_np=np1)
    while urgent:
        urgent.popleft()()
    while queue:
        queue.popleft()()


_PROGRAM = None


def _build_program():
    nc = bacc.Bacc(
        "TRN2", target_bir_lowering=False, debug=False,
        enable_asserts=False, num_devices=N_CORES,
    )
    xTb = nc.dram_tensor("xTb", [D, BT], BF, kind="ExternalInput").ap()
    wqT = nc.dram_tensor("wqT", [D, P], BF, kind="ExternalInput").ap()
    wkT = nc.dram_tensor("wkT", [D, P], BF, kind="ExternalInput").ap()
    wvT = nc.dram_tensor("wvT", [D, P], BF, kind="ExternalInput").ap()
    woT = nc.dram_tensor("woT", [P, D], BF, kind="ExternalInput").ap()
    ident_in = nc.dram_tensor("ident_in", [P, P], F32, kind="ExternalInput").ap()
    ones_in = nc.dram_tensor("ones_in", [P, NS * 65], BF,
                             kind="ExternalInput").ap()
    sel_in = nc.dram_tensor("sel_in", [2, P], F32R, kind="ExternalInput").ap()
    out = nc.dram_tensor("out", [BT, D], BF, kind="ExternalOutput").ap()
    with tile.TileContext(nc) as tc:
        _mhsa_kernel(tc, out, xTb, wqT, wkT, wvT, woT, ident_in, ones_in,
                     sel_in)
    nc.compile()
    return nc


def get_program():
    global _PROGRAM
    if _PROGRAM is None:
        _PROGRAM = _build_program()
    return _PROGRAM


last_results = None


def _install_trace_hook():
    """Register the axon NTFF-profile hook that the agent image's antenv
    lacks, so run_bass_kernel_spmd(trace=True) can capture HW timings."""
    import sys
    import types

    if "antenv.axon_hooks" in sys.modules:
        return
    try:
        from trn_agent_boot.trn_boot import _ntff_profile_via_ctypes
        hook = _ntff_profile_via_ctypes("/opt/axon/libaxon_pjrt.so")
    except Exception:
        hook = None
    mod = types.ModuleType("antenv.axon_hooks")
    state = {"hook": hook}
    mod.get_axon_ntff_profile_hook = lambda: state["hook"]
    mod.set_axon_ntff_profile_hook = lambda h: state.__setitem__("hook", h)
    sys.modules["antenv.axon_hooks"] = mod

    import concourse.bass_utils as bu
    orig_upload = bu.upload_artifacts

    def safe_upload(tmpdir):
        try:
            return orig_upload(tmpdir)
        except Exception:
            return tmpdir

    bu.upload_artifacts = safe_upload


def kernel(x, W_q, W_k, W_v, W_o):
    global last_results
    import ml_dtypes
    bf16 = ml_dtypes.bfloat16

    x = np.ascontiguousarray(np.asarray(x, dtype=np.float32))
    W_q = np.asarray(W_q, dtype=np.float32)
    W_k = np.asarray(W_k, dtype=np.float32)
    W_v = np.asarray(W_v, dtype=np.float32)
    W_o = np.asarray(W_o, dtype=np.float32)

    xTb = np.ascontiguousarray(x.reshape(BT, D).T.astype(bf16))
    ident = np.eye(P, dtype=np.float32)
    ones_arr = np.ones((P, NS * 65), dtype=bf16)
    sel = np.zeros((2, P), dtype=np.float32)
    sel[0, 0:64] = 1.0
    sel[1, 64:128] = 1.0
    in_maps = []
    for c in range(N_CORES):
        sl = slice(P * c, P * (c + 1))
        in_maps.append({
            "xTb": xTb,
            "wqT": np.ascontiguousarray(W_q[sl, :].T.astype(bf16)),
            "wkT": np.ascontiguousarray(W_k[sl, :].T.astype(bf16)),
            "wvT": np.ascontiguousarray(W_v[sl, :].T.astype(bf16)),
            "woT": np.ascontiguousarray(W_o[:, sl].T.astype(bf16)),
            "ident_in": ident,
            "ones_in": ones_arr,
            "sel_in": sel,
        })

    trace = bool(int(os.environ.get("KERNEL_TRACE", "0")))
    if trace:
        _install_trace_hook()
    nc = get_program()
    res = run_bass_kernel_spmd(
        nc, in_maps, core_ids=list(range(N_CORES)), trace=trace,
    )
    last_results = res
    total = res.results[0]["out"].astype(np.float32)
    for r in res.results[1:]:
        total = total + r["out"].astype(np.float32)
    return total.reshape(NB, T, D)


# revision 49
# speedup vs baseline: 1.1373x; 1.1373x over previous
"""Multi-head self-attention on 8 Trainium2 NeuronCores.

Problem: x[2, 2048, 1024], 16 heads, Dh=64, fp32.
  q/k/v = x @ W.T ; scores = q k^T / 8 ; out = softmax(scores) v @ W_o.T

Sharding (Megatron-style): each core owns 2 heads (128 of the 1024 model
dims). W_q/W_k/W_v column-sharded, W_o row-sharded; the cross-core
all-reduce of the output-projection partials is done on the host.

Per-core design (v2, all-bf16 matmuls):
  - Every matmul operand is bf16: 1 column/cycle on the PE at 2.4 GHz and
    fast-weight-load stays enabled (fp32/fp32r HIGH mode disables FWL and
    measured ~1.3-1.5 cyc/col). PSUM accumulation is fp32 throughout.
  - x is fed pre-transposed (xT [1024, 4096] bf16); one 3D-pattern DMA per
    512-token block gathers all 8 contraction strips into one SBUF tile.
  - qT/kT computed as [128 = 2 heads x 64, 4096] (head dims on partitions).
  - Scores computed transposed, S^T[k_tok, q_tok], both heads in one
    [128, 1024] 2-bank PSUM tile per k-strip; exp() on ScalarE reads PSUM
    directly (PSUM access is cheaper than SBUF for ACT) and writes bf16.
  - v transposed to natural layout via PE transpose, augmented with a ones
    column so the PV matmul (M=65) emits softmax denominators in the same
    accumulation chain.
  - Denominator path stays on-chip: DVE reciprocal (fp22) on the PSUM
    denominator rows, one tiny SBUF->SBUF DMA to rebase the two rows onto
    partitions 0/1, then a single K=2 matmul broadcasts 1/den across all
    128 partitions for the normalization multiply.
  - Output written bf16; the host all-reduce upcasts to fp32.
  - Software-pipelined schedule: ScalarE runs one exp [128,1024] per
    k-strip back-to-back; all PE work (scores, PV, QKV chains, transposes,
    output projection) is interleaved as filler chunks between strips.
"""

import os
from contextlib import ExitStack

import numpy as np

import concourse.bass as bass
import concourse.tile as tile
from concourse import bacc, mybir
from concourse._compat import with_exitstack
from concourse.bass_utils import run_bass_kernel_spmd

F32 = mybir.dt.float32
F32R = mybir.dt.float32r
BF = mybir.dt.bfloat16
EXP = mybir.ActivationFunctionType.Exp

P = 128          # partitions / head-pair dims per core
D = 1024         # model dim
T = 2048         # tokens per batch
NB = 2           # batches
BT = NB * T      # 4096 flattened tokens
KT = D // P      # 8 contraction tiles over model dim
NQ = T // 512    # 4 q-tiles of 512 per batch
NS = T // P      # 16 k-strips of 128 per batch
N_CORES = 8


@with_exitstack
def _mhsa_kernel(ctx: ExitStack, tc: tile.TileContext, out, xTb, wqT, wkT,
                 wvT, woT, ident_in, ones_in):
    nc = tc.nc

    # ---- pools ----
    wpool = ctx.enter_context(tc.tile_pool(name="weights", bufs=1))
    xpool = ctx.enter_context(tc.tile_pool(name="xtiles", bufs=3))
    qkpool = ctx.enter_context(tc.tile_pool(name="qk", bufs=1))
    vtpool = ctx.enter_context(tc.tile_pool(name="vt", bufs=2))
    vapool = ctx.enter_context(tc.tile_pool(name="vaug", bufs=4))
    expool = ctx.enter_context(tc.tile_pool(name="expp", bufs=4))
    aupool = ctx.enter_context(tc.tile_pool(name="avun", bufs=3))
    anpool = ctx.enter_context(tc.tile_pool(name="avnorm", bufs=4))
    rpool = ctx.enter_context(tc.tile_pool(name="recip", bufs=2))
    opool = ctx.enter_context(tc.tile_pool(name="outsb", bufs=4))
    drpool = ctx.enter_context(tc.tile_pool(name="dscr", bufs=4, space="DRAM"))

    ps_sc = ctx.enter_context(tc.tile_pool(name="ps_sc", bufs=2, space="PSUM"))
    ps_pv = ctx.enter_context(tc.tile_pool(name="ps_pv", bufs=2, space="PSUM"))
    ps_misc = ctx.enter_context(tc.tile_pool(name="ps_misc", bufs=2,
                                             space="PSUM"))

    # ---- weights / constants (resident) ----
    # One DMA per projection: wqT [1024, 128] -> [128, 8*128] with the 8
    # contraction strips side by side on the free axis.
    wq_sb = wpool.tile([P, KT * P], BF, name="wq_sb")
    wk_sb = wpool.tile([P, KT * P], BF, name="wk_sb")
    wv_sb = wpool.tile([P, KT * P], BF, name="wv_sb")
    wo_sb = wpool.tile([P, D], BF, name="wo_sb")
    ident = wpool.tile([P, P], F32, name="ident")
    for eng, w_sb, w_in in ((nc.sync, wq_sb, wqT), (nc.gpsimd, wk_sb, wkT),
                            (nc.sync, wv_sb, wvT)):
        eng.dma_start(
            out=w_sb[:].rearrange("p (j m) -> p j m", j=KT),
            in_=w_in.rearrange("(j p) m -> p j m", p=P),
        )
    nc.gpsimd.dma_start(out=wo_sb[:], in_=woT[:])
    nc.sync.dma_start(out=ident[:], in_=ident_in[:])

    qT = qkpool.tile([P, BT], BF, name="qT")
    kTt = qkpool.tile([P, BT], BF, name="kTt")

    va = {}       # (b, 'A'/'B') -> augmented-v tile, (b, 'T') -> vT staging
    avs = {}      # (b, n, 'A'/'B') -> PV psum accumulators
    xts = {}      # (b, n) -> x strip tile [128, 8*512]

    def emit_qkv_chunks(b, inline_n0=False):
        """Chunk list: x loads, QKV chains, v transposes for batch b.

        Chains are split in two 4-matmul halves so a single chunk never
        occupies the PE for more than ~1us between attention strips.
        """
        chunks = []

        def init():
            vA = vapool.tile([P, NS * 65], BF, name=f"vA{b}", tag="va")
            vB = vapool.tile([P, NS * 65], BF, name=f"vB{b}", tag="va")
            nc.gpsimd.dma_start(out=vA[:], in_=ones_in[:])
            nc.gpsimd.dma_start(out=vB[:], in_=ones_in[:])
            va[(b, "A")] = vA
            va[(b, "B")] = vB
            vT = vtpool.tile([P, T], F32, name=f"vT{b}", tag="vt")
            va[(b, "T")] = vT
        chunks.append(init)

        def load_n(n):
            def f():
                col = b * T + n * 512
                xt = xpool.tile([P, KT * 512], BF, name=f"xt{b}_{n}", tag="xt")
                nc.sync.dma_start(
                    out=xt[:].rearrange("p (j c) -> p j c", j=KT),
                    in_=xTb.rearrange("(j p) c -> p j c", p=P)[:, :, col:col + 512],
                )
                xts[(b, n)] = xt
            return f

        def chain(n, which):
            # One chunk per chain: every reader of the PSUM accumulator is
            # emitted in-chunk, so interleaved chunks can never recycle it
            # out from under a not-yet-emitted reader.
            def f():
                acc = ps_misc.tile([P, 512], F32, name="qkv_ps", tag="mps")
                w_sb = {"q": wq_sb, "k": wk_sb, "v": wv_sb}[which]
                xt = xts[(b, n)]
                for j in range(KT):
                    nc.tensor.matmul(
                        acc[:], w_sb[:, j * P:(j + 1) * P],
                        xt[:, j * 512:(j + 1) * 512],
                        start=(j == 0), stop=(j == KT - 1),
                    )
                dst, col = {
                    "q": (qT, b * T + n * 512),
                    "k": (kTt, b * T + n * 512),
                    "v": (va[(b, "T")], n * 512),
                }[which]
                nc.vector.tensor_copy(dst[:, col:col + 512], acc[:])
            return f

        def trans(s):
            def f():
                vT = va[(b, "T")]
                tp = ps_misc.tile([P, P], F32, name="tr_ps", tag="mps")
                nc.tensor.transpose(tp[:], vT[:, s * P:(s + 1) * P], ident[:])
                nc.vector.tensor_copy(
                    va[(b, "A")][:, s * 65:s * 65 + 64], tp[:, 0:64])
                nc.vector.tensor_copy(
                    va[(b, "B")][:, s * 65:s * 65 + 64], tp[:, 64:128])
            return f

        # Per-n interleave keeps the 3-deep x-tile pool cycling: all of
        # xt(n)'s readers (k/v/q chains) are emitted before load(n+3)
        # needs its slot back, so the in-order queues can't cycle.
        chunks.append(init)
        for n in range(NQ):
            chunks.append(load_n(n))
            for which in ("k", "v", "q"):
                chunks.append(chain(n, which))
            for s in range(4 * n, 4 * n + 4):
                chunks.append(trans(s))
        return chunks

    def make_norm_outproj(b):
        """Per-q-tile deferred closure lists: normalize + project.

        The PSUM readers (av_un copies, reciprocals) are emitted INLINE at
        the end of each q-tile by `den_inline` — deferring them past the
        next q-tile's PSUM-tile allocations would hand their buffers out
        before the reads are visible to the dependency tracker.
        """
        state = {}

        def den_inline(n):
            # Reciprocal cost on DVE scales with free-size PER LANE, so the
            # two [1,512] denominator rows are bounced through DRAM into a
            # [128,8] gather (8 elems/lane) before inverting, then scattered
            # back for the partition-broadcast loads in norm(). No PE work
            # anywhere in this chain — nothing ahead of the next q-tile's
            # scores can stall on it.
            avA = avs[(b, n, "A")]
            avB = avs[(b, n, "B")]
            av_un = aupool.tile([P, 512], BF, name="av_un", tag="aun")
            nc.vector.tensor_copy(av_un[0:64, :], avA[0:64, :])
            nc.vector.tensor_copy(av_un[64:128, :], avB[0:64, :])
            drow = rpool.tile([65, 1024], F32R, name="drow", tag="drow")
            nc.vector.tensor_copy(drow[64:65, 0:512], avA[64:65, :])
            nc.vector.tensor_copy(drow[64:65, 512:1024], avB[64:65, :])
            dscr = drpool.tile([1, 1024], F32R, name="dscr", tag="dscr")
            rscr = drpool.tile([1, 1024], F32R, name="rscr", tag="rscr")
            nc.sync.dma_start(out=dscr[:], in_=drow[64:65, :])
            rin = rpool.tile([P, 8], F32R, name="rin", tag="rin")
            rout = rpool.tile([P, 8], F32R, name="rout", tag="rout")
            nc.sync.dma_start(
                out=rin[:], in_=dscr.rearrange("p (a c) -> (p a) c", a=P))
            with nc.allow_low_precision(reason="fp22 softmax denom"):
                nc.vector.reciprocal(rout[:], rin[:])
            nc.sync.dma_start(
                out=rscr.rearrange("p (a c) -> (p a) c", a=P), in_=rout[:])
            state[(n, "un")] = av_un
            state[(n, "rs")] = rscr

        def norm(n):
            def f():
                rscr = state[(n, "rs")]
                bc_sb = anpool.tile([P, 512], F32R, name="bc_sb", tag="bcs")
                nc.gpsimd.dma_start(
                    out=bc_sb[0:64, :],
                    in_=rscr[0:1, 0:512].to_broadcast((64, 512)))
                nc.gpsimd.dma_start(
                    out=bc_sb[64:128, :],
                    in_=rscr[0:1, 512:1024].to_broadcast((64, 512)))
                av_n = anpool.tile([P, 512], BF, name="av_n", tag="avn")
                nc.vector.tensor_mul(av_n[:], state[(n, "un")][:], bc_sb[:])
                state[(n, "nrm")] = av_n
            return f

        def outproj(n, sub):
            def f():
                av_n = state[(n, "nrm")]
                row0 = b * T + n * 512 + sub * P
                for jh in range(2):
                    op = ps_misc.tile([P, 512], F32, name="op_ps", tag="mps")
                    nc.tensor.matmul(
                        op[:],
                        av_n[:, sub * P:(sub + 1) * P],
                        wo_sb[:, jh * 512:(jh + 1) * 512],
                        start=True, stop=True,
                    )
                    ot = opool.tile([P, 512], BF, name="ot", tag="ot")
                    nc.vector.tensor_copy(ot[:], op[:])
                    eng = nc.sync if (sub + jh) % 2 == 0 else nc.gpsimd
                    eng.dma_start(
                        out=out[row0:row0 + P, jh * 512:(jh + 1) * 512],
                        in_=ot[:],
                    )
            return f

        def per_n(n):
            return [norm(n)] + [outproj(n, sub) for sub in range(4)]
        return den_inline, per_n

    def attn_batch(b, queue, urgent, self_np=None):
        """Batch-b attention; filler chunks drained between strip steps.

        `urgent` chunks (deferred norm/outproj of a finished q-tile) pop
        ahead of regular fillers so their tile-pool buffers recycle on
        schedule. Leftovers are the caller's to drain.
        """
        den_inline = None
        if self_np is not None:
            den_inline, per_n = self_np
        for n in range(NQ):
            qcol = b * T + n * 512
            avA = ps_pv.tile([P, 512], F32, name="avA", tag="pv")
            avB = ps_pv.tile([P, 512], F32, name="avB", tag="pv")
            avs[(b, n, "A")] = avA
            avs[(b, n, "B")] = avB
            for s in range(NS):
                kcol = b * T + s * P
                sc = ps_sc.tile([P, 1024], F32, name="sc", tag="sc")
                nc.tensor.matmul(
                    sc[:, 0:512],
                    kTt[0:64, kcol:kcol + P],
                    qT[0:64, qcol:qcol + 512],
                    start=True, stop=True,
                )
                nc.tensor.matmul(
                    sc[:, 512:1024],
                    kTt[64:128, kcol:kcol + P],
                    qT[64:128, qcol:qcol + 512],
                    start=True, stop=True,
                )
                ex = expool.tile([P, 1024], BF, name="ex", tag="ex")
                nc.scalar.activation(out=ex[:], in_=sc[:], func=EXP,
                                     scale=0.125)
                # PE filler work goes here, during the exp wait.
                if urgent:
                    urgent.popleft()()
                elif queue:
                    queue.popleft()()
                nc.tensor.matmul(
                    avA[0:65, :],
                    va[(b, "A")][:, s * 65:(s + 1) * 65],
                    ex[:, 0:512],
                    start=(s == 0), stop=(s == NS - 1),
                )
                nc.tensor.matmul(
                    avB[0:65, :],
                    va[(b, "B")][:, s * 65:(s + 1) * 65],
                    ex[:, 512:1024],
                    start=(s == 0), stop=(s == NS - 1),
                )
            if self_np is not None:
                den_inline(n)
                urgent.extend(per_n(n))

    # ---- software-pipelined schedule ----
    from collections import deque
    chunks0 = emit_qkv_chunks(0)
    # Inline: everything attn(b0) strips read (x, k, v, q, transposes).
    # Batch 1's chunks become the fillers inside batch 0's attention.
    for c in chunks0:
        c()
    np0 = make_norm_outproj(0)
    np1 = make_norm_outproj(1)
    urgent = deque()
    queue = deque(emit_qkv_chunks(1))
    attn_batch(0, queue, urgent, self_np=np0)
    attn_batch(1, queue, urgent, self_np=np1)
    while urgent:
        urgent.popleft()()
    while queue:
        queue.popleft()()


_PROGRAM = None


def _build_program():
    nc = bacc.Bacc(
        "TRN2", target_bir_lowering=False, debug=False,
        enable_asserts=False, num_devices=N_CORES,
    )
    xTb = nc.dram_tensor("xTb", [D, BT], BF, kind="ExternalInput").ap()
    wqT = nc.dram_tensor("wqT", [D, P], BF, kind="ExternalInput").ap()
    wkT = nc.dram_tensor("wkT", [D, P], BF, kind="ExternalInput").ap()
    wvT = nc.dram_tensor("wvT", [D, P], BF, kind="ExternalInput").ap()
    woT = nc.dram_tensor("woT", [P, D], BF, kind="ExternalInput").ap()
    ident_in = nc.dram_tensor("ident_in", [P, P], F32, kind="ExternalInput").ap()
    ones_in = nc.dram_tensor("ones_in", [P, NS * 65], BF,
                             kind="ExternalInput").ap()
    out = nc.dram_tensor("out", [BT, D], BF, kind="ExternalOutput").ap()
    with tile.TileContext(nc) as tc:
        _mhsa_kernel(tc, out, xTb, wqT, wkT, wvT, woT, ident_in, ones_in)
    nc.compile()
    return nc


def get_program():
    global _PROGRAM
    if _PROGRAM is None:
        _PROGRAM = _build_program()
    return _PROGRAM


last_results = None


def _install_trace_hook():
    """Register the axon NTFF-profile hook that the agent image's antenv
    lacks, so run_bass_kernel_spmd(trace=True) can capture HW timings."""
    import sys
    import types

    if "antenv.axon_hooks" in sys.modules:
        return
    try:
        from trn_agent_boot.trn_boot import _ntff_profile_via_ctypes
        hook = _ntff_profile_via_ctypes("/opt/axon/libaxon_pjrt.so")
    except Exception:
        hook = None
    mod = types.ModuleType("antenv.axon_hooks")
    state = {"hook": hook}
    mod.get_axon_ntff_profile_hook = lambda: state["hook"]
    mod.set_axon_ntff_profile_hook = lambda h: state.__setitem__("hook", h)
    sys.modules["antenv.axon_hooks"] = mod

    import concourse.bass_utils as bu
    orig_upload = bu.upload_artifacts

    def safe_upload(tmpdir):
        try:
            return orig_upload(tmpdir)
        except Exception:
            return tmpdir

    bu.upload_artifacts = safe_upload


def kernel(x, W_q, W_k, W_v, W_o):
    global last_results
    import ml_dtypes
    bf16 = ml_dtypes.bfloat16

    x = np.ascontiguousarray(np.asarray(x, dtype=np.float32))
    W_q = np.asarray(W_q, dtype=np.float32)
    W_k = np.asarray(W_k, dtype=np.float32)
    W_v = np.asarray(W_v, dtype=np.float32)
    W_o = np.asarray(W_o, dtype=np.float32)

    xTb = np.ascontiguousarray(x.reshape(BT, D).T.astype(bf16))
    ident = np.eye(P, dtype=np.float32)
    ones_arr = np.ones((P, NS * 65), dtype=bf16)
    in_maps = []
    for c in range(N_CORES):
        sl = slice(P * c, P * (c + 1))
        in_maps.append({
            "xTb": xTb,
            "wqT": np.ascontiguousarray(W_q[sl, :].T.astype(bf16)),
            "wkT": np.ascontiguousarray(W_k[sl, :].T.astype(bf16)),
            "wvT": np.ascontiguousarray(W_v[sl, :].T.astype(bf16)),
            "woT": np.ascontiguousarray(W_o[:, sl].T.astype(bf16)),
            "ident_in": ident,
            "ones_in": ones_arr,
        })

    trace = bool(int(os.environ.get("KERNEL_TRACE", "0")))
    if trace:
        _install_trace_hook()
    nc = get_program()
    res = run_bass_kernel_spmd(
        nc, in_maps, core_ids=list(range(N_CORES)), trace=trace,
    )
    last_results = res
    total = res.results[0]["out"].astype(np.float32)
    for r in res.results[1:]:
        total = total + r["out"].astype(np.float32)
    return total.reshape(NB, T, D)
